# revision 8
# baseline (speedup 1.0000x reference)
"""Trainium2 Bass kernel: multi-head FAVOR+ attention with multi-scale
random-Fourier-feature inputs and rotary position embeddings.

Sharding: data-parallel over batch. Each of the 8 NeuronCores processes one
batch element end-to-end (every head's phi(q)/phi(k)/kv computation is
independent per batch element, so no collectives are needed); the host
scatters x over cores and stacks the per-core outputs.

Self-contained: hardcodes all shapes from the problem spec.
"""

import math

import numpy as np

B, S, D = 8, 4096, 768
H, HD = 8, 96
M = 256
R = 128
BASE = 12315
P = 128                  # tokens per chunk (= SBUF partitions)
NCH = S // P             # 32 chunks
ND = D // P              # 6 feature chunks of 128
TWO_PI = 2.0 * math.pi
C = math.sqrt(2.0 / M)   # multiscale feature scale
BOFF = 8.0
MAGIC = 8388608.0         # 2^23: (t + MAGIC) - MAGIC == round(t) for 0<t<2^22               # positivity offset so mod(x, 1) is period-exact

_cache = {}


def _build(s_total, n_cores):
    """Build + compile the Bass module for one core processing [s_total, D]."""
    import concourse.bass as bass  # noqa: F401
    import concourse.tile as tile
    from concourse import bacc, mybir
    from concourse.masks import make_identity

    dt = mybir.dt
    AF = mybir.ActivationFunctionType
    ALU = mybir.AluOpType
    f32, bf16 = dt.float32, dt.bfloat16
    nch = s_total // P

    nc = bacc.Bacc(
        "TRN2", target_bir_lowering=False, debug=False, num_devices=n_cores
    )

    x_d = nc.dram_tensor("x", [s_total, D], f32, kind="ExternalInput").ap()
    wq_d = nc.dram_tensor("wq", [P, ND, D], f32, kind="ExternalInput").ap()
    wk_d = nc.dram_tensor("wk", [P, ND, D], f32, kind="ExternalInput").ap()
    wv_d = nc.dram_tensor("wv", [P, ND, D], f32, kind="ExternalInput").ap()
    wo_d = nc.dram_tensor("wo", [P, ND, D], f32, kind="ExternalInput").ap()
    bq_d = nc.dram_tensor("bq", [2, D], f32, kind="ExternalInput").ap()
    bk_d = nc.dram_tensor("bk", [2, D], f32, kind="ExternalInput").ap()
    bv_d = nc.dram_tensor("bv", [2, D], f32, kind="ExternalInput").ap()
    ob_d = nc.dram_tensor("ob", [P, D], f32, kind="ExternalInput").ap()
    rc_d = nc.dram_tensor("rc", [s_total, 48], f32, kind="ExternalInput").ap()
    rs_d = nc.dram_tensor("rs", [s_total, 96], f32, kind="ExternalInput").ap()
    fw_d = nc.dram_tensor("fw", [96, R], f32, kind="ExternalInput").ap()
    fwk_d = nc.dram_tensor("fwk", [98, R], f32, kind="ExternalInput").ap()
    fb_d = nc.dram_tensor("fb", [R, 1], f32, kind="ExternalInput").ap()
    out_d = nc.dram_tensor("out", [s_total, D], f32, kind="ExternalOutput").ap()

    with tile.TileContext(nc) as tc:
        with tc.tile_pool(name="const", bufs=1) as const:
            wq = const.tile([P, ND, D], bf16)
            nc.gpsimd.dma_start(out=wq, in_=wq_d)
            wk = const.tile([P, ND, D], bf16)
            nc.gpsimd.dma_start(out=wk, in_=wk_d)
            wv = const.tile([P, ND, D], bf16)
            nc.gpsimd.dma_start(out=wv, in_=wv_d)
            wo = const.tile([P, ND, D], bf16)
            nc.gpsimd.dma_start(out=wo, in_=wo_d)
            bq = const.tile([2, D], bf16)
            nc.gpsimd.dma_start(out=bq, in_=bq_d)
            bk = const.tile([2, D], bf16)
            nc.gpsimd.dma_start(out=bk, in_=bk_d)
            bv = const.tile([2, D], bf16)
            nc.gpsimd.dma_start(out=bv, in_=bv_d)
            ones1 = const.tile([2, P], bf16)
            nc.vector.memset(ones1, 1.0)
            ob = const.tile([P, D], f32)
            nc.sync.dma_start(out=ob, in_=ob_d)
            fw = const.tile([96, R], bf16)
            nc.gpsimd.dma_start(out=fw, in_=fw_d)
            fwk = const.tile([98, R], bf16)
            nc.gpsimd.dma_start(out=fwk, in_=fwk_d)
            fb = const.tile([R, 1], f32)
            nc.sync.dma_start(out=fb, in_=fb_d)
            idb = const.tile([P, P], bf16)
            make_identity(nc, idb)
            idf = const.tile([P, P], f32)
            make_identity(nc, idf)
            # resident q' features (feature-major) for pass 2
            qpT = const.tile([P, H, nch, P], bf16)
            kvaug = const.tile([P, H, 97], bf16)
            qT = const.tile([P, H, P], bf16)
            kT = const.tile([P, H, P], bf16)
            nc.vector.memset(kT[96:98, :, :], 1.0)
            kvsb = const.tile([P, H * R], f32)

            biases = {"q": bq, "k": bk, "v": bv}
            weights = {"q": wq, "k": wk, "v": wv}

            with (
                tc.tile_pool(name="w1", bufs=2) as w1,
                tc.tile_pool(name="w2", bufs=3) as w2,
                tc.tile_pool(name="bigps", bufs=2, space="PSUM") as bigps,
                tc.tile_pool(name="tps", bufs=2, space="PSUM") as tps,
                tc.tile_pool(name="kvps", bufs=1, space="PSUM") as kvps,
            ):
                kv_ps = kvps.tile([P, H * R], f32)
                for ci in range(nch):
                    sl = slice(ci * P, (ci + 1) * P)
                    xb = w1.tile([P, D], bf16, tag="xb")
                    nc.gpsimd.dma_start(out=xb, in_=x_d[sl, :])
                    rc = w1.tile([P, 48], f32, tag="rc")
                    nc.sync.dma_start(out=rc, in_=rc_d[sl, :])
                    rs = w1.tile([P, 96], f32, tag="rs")
                    nc.sync.dma_start(out=rs, in_=rs_d[sl, :])

                    # xT chunks for the multiscale contraction over D
                    XT = w1.tile([P, ND, P], bf16, tag="XT")
                    for j in range(ND):
                        pt = tps.tile([P, P], bf16, tag="pt")
                        nc.tensor.transpose(pt, xb[:, j * P : (j + 1) * P], idb)
                        if j % 2 == 0:
                            nc.scalar.copy(XT[:, j, :], pt)
                        else:
                            nc.vector.tensor_copy(XT[:, j, :], pt)

                    feats = {}
                    for name in ("q", "k", "v"):
                        w_sb = weights[name]
                        pm = bigps.tile([P, H * R], f32, tag="big")
                        b_sb = biases[name]
                        nc.tensor.matmul(
                            pm[:, 0:512], ones1, b_sb[:, 0:512],
                            start=True, stop=False,
                        )
                        nc.tensor.matmul(
                            pm[:, 512:768], ones1, b_sb[:, 512:768],
                            start=True, stop=False,
                        )
                        for j in range(ND):
                            nc.tensor.matmul(
                                pm[:, 0:512],
                                XT[:, j, :],
                                w_sb[:, j, 0:512],
                                start=False,
                                stop=(j == ND - 1),
                            )
                            nc.tensor.matmul(
                                pm[:, 512:768],
                                XT[:, j, :],
                                w_sb[:, j, 512:768],
                                start=False,
                                stop=(j == ND - 1),
                            )
                        # r = t - round(t) in [-0.5, 0.5]; sin(2*pi*r) is the
                        # range-reduced multiscale cosine feature
                        t2 = w2.tile([P, D], f32, tag="t2")
                        nc.vector.tensor_scalar(
                            out=t2, in0=pm[:, 0:D], scalar1=MAGIC, scalar2=MAGIC,
                            op0=ALU.add, op1=ALU.subtract,
                        )
                        t1 = w2.tile([P, D], f32, tag="t1")
                        nc.vector.tensor_sub(t1, pm[:, 0:D], t2)
                        if name == "v":
                            vaug = w1.tile([P, H, 97], bf16, tag="vaug")
                            nc.scalar.activation(
                                vaug[:, :, 0:96],
                                t1.rearrange("p (h f) -> p h f", h=H),
                                AF.Sin,
                                bias=0.0,
                                scale=TWO_PI,
                            )
                            nc.vector.memset(vaug[:, :, 96:97], 1.0)
                            feats[name] = vaug
                        else:
                            ft = w1.tile([P, D], bf16, tag="f" + name)
                            nc.scalar.activation(
                                ft, t1, AF.Sin, bias=0.0, scale=TWO_PI
                            )
                            feats[name] = ft

                    # -0.5*||k_h||^2 (rotation-invariant: use pre-rotary feats)
                    k2 = w2.tile([P, D], f32, tag="k2")
                    nc.vector.tensor_mul(k2, feats["k"], feats["k"])
                    nsq = w2.tile([P, H], f32, tag="nsq")
                    nc.vector.reduce_sum(
                        nsq,
                        k2.rearrange("p (h f) -> p h f", h=H),
                        axis=mybir.AxisListType.X,
                    )
                    nc.vector.tensor_scalar_mul(nsq, nsq, -0.5 * C * C)

                    # rotary (multipliers carry the sqrt(2/M) feature scale)
                    rc_b = rc[:, None, None, :].broadcast_to([P, H, 2, 48])
                    rs_b = rs.rearrange("p (t f) -> p t f", t=2)[
                        :, None, :, :
                    ].broadcast_to([P, H, 2, 48])
                    rots = {}
                    for name in ("q", "k"):
                        f4 = feats[name].rearrange(
                            "p (h t f) -> p h t f", h=H, t=2
                        )
                        sw = f4[:, :, ::-1, :]
                        ra = w2.tile([P, H, 2, 48], bf16, tag="ra")
                        nc.vector.tensor_mul(ra, f4, rc_b)
                        rb = w2.tile([P, H, 2, 48], bf16, tag="rb")
                        nc.vector.tensor_mul(rb, sw, rs_b)
                        ro = w1.tile([P, D], bf16, tag="ro" + name)
                        nc.vector.tensor_add(
                            ro.rearrange("p (h t f) -> p h t f", h=H, t=2), ra, rb
                        )
                        rots[name] = ro

                    # per-head feature-major views of rotated q/k
                    for h in range(H):
                        ptq = tps.tile([P, P], bf16, tag="pt")
                        nc.tensor.transpose(
                            ptq[0:96, :], rots["q"][:, h * 96 : (h + 1) * 96], idb
                        )
                        if h % 2 == 0:
                            nc.scalar.copy(qT[0:96, h, :], ptq[0:96, :])
                        else:
                            nc.vector.tensor_copy(qT[0:96, h, :], ptq[0:96, :])
                        ptk = tps.tile([P, P], bf16, tag="pt")
                        nc.tensor.transpose(
                            ptk[0:96, :], rots["k"][:, h * 96 : (h + 1) * 96], idb
                        )
                        if h % 2 == 1:
                            nc.scalar.copy(kT[0:96, h, :], ptk[0:96, :])
                        else:
                            nc.vector.tensor_copy(kT[0:96, h, :], ptk[0:96, :])

                    # q' = exp(q.W + b); the exp(-0.5||q||^2) factor cancels in
                    # the final ratio (it scales numerator and z identically)
                    pfq = bigps.tile([P, H * R], f32, tag="big")
                    for h in range(H):
                        nc.tensor.matmul(
                            pfq[:, h * P : (h + 1) * P],
                            fw,
                            qT[0:96, h, :],
                            start=True,
                            stop=True,
                        )
                    nc.scalar.activation(
                        qpT[:, :, ci, :],
                        pfq.rearrange("p (h s) -> p h s", h=H),
                        AF.Exp,
                        bias=fb,
                        scale=1.0,
                    )

                    # k' = exp(k.W + b - 0.5||k||^2), token-major
                    pfk = bigps.tile([P, H * R], f32, tag="big")
                    for h in range(H):
                        nc.tensor.matmul(
                            pfk[:, h * R : (h + 1) * R],
                            kT[0:98, h, :],
                            fwk,
                            start=True,
                            stop=True,
                        )
                    pfk3 = pfk.rearrange("p (h r) -> p h r", h=H)
                    nc.vector.tensor_add(
                        pfk3, pfk3, nsq[:, :, None].broadcast_to([P, H, R])
                    )
                    kp = w1.tile([P, H, R], bf16, tag="kp")
                    nc.scalar.activation(kp, pfk3, AF.Exp)

                    # kv[r, hd] accumulation; v is augmented with a ones
                    # column so row 96 accumulates ksum[r]
                    for h in range(H):
                        nc.tensor.matmul(
                            kv_ps[0:97, h * R : (h + 1) * R],
                            feats["v"][:, h, :],
                            kp[:, h, :],
                            start=(ci == 0 and h % 4 == 0),
                            stop=(ci == nch - 1 and h % 4 == 3),
                        )

                # [kv | ksum] -> [r, 97] per head, bf16
                nc.vector.tensor_copy(kvsb[0:97, :], kv_ps[0:97, :])
                for h in range(H):
                    pt = tps.tile([P, P], f32, tag="pt")
                    nc.tensor.transpose(
                        pt[:, 0:97],
                        kvsb[0:97, h * R : (h + 1) * R],
                        idf[0:97, 0:97],
                    )
                    if h % 2 == 0:
                        nc.scalar.copy(kvaug[:, h, :], pt[:, 0:97])
                    else:
                        nc.vector.tensor_copy(kvaug[:, h, :], pt[:, 0:97])

            # ---- pass 2: attention readout + output projection ----
            with (
                tc.tile_pool(name="p2s", bufs=2) as p2s,
                tc.tile_pool(name="atps", bufs=1, space="PSUM") as atps,
                tc.tile_pool(name="ops", bufs=2, space="PSUM") as ops,
                tc.tile_pool(name="tps2", bufs=2, space="PSUM") as tps2,
            ):
                for ci in range(nch):
                    pat = atps.tile([P, H * P], f32, tag="at")
                    for h in range(H):
                        nc.tensor.matmul(
                            pat[:, h * P : h * P + 97],
                            qpT[:, h, ci, :],
                            kvaug[:, h, :],
                            start=True,
                            stop=True,
                        )
                    pat3 = pat.rearrange("p (h r) -> p h r", h=H)
                    rz = p2s.tile([P, H], f32, tag="rz")
                    nc.vector.reciprocal(rz, pat3[:, :, 96])
                    at = p2s.tile([P, D], bf16, tag="at")
                    nc.vector.tensor_mul(
                        at.rearrange("p (h f) -> p h f", h=H),
                        pat3[:, :, 0:96],
                        rz[:, :, None].broadcast_to([P, H, 96]),
                    )
                    aT = p2s.tile([P, ND, P], bf16, tag="aT")
                    for j in range(ND):
                        pt = tps2.tile([P, P], bf16, tag="pt2")
                        nc.tensor.transpose(pt, at[:, j * P : (j + 1) * P], idb)
                        if j % 2 == 0:
                            nc.scalar.copy(aT[:, j, :], pt)
                        else:
                            nc.vector.tensor_copy(aT[:, j, :], pt)
                    po = ops.tile([P, D], f32, tag="po")
                    for j in range(ND):
                        nc.tensor.matmul(
                            po[:, 0:512],
                            aT[:, j, :],
                            wo[:, j, 0:512],
                            start=(j == 0),
                            stop=(j == ND - 1),
                        )
                        nc.tensor.matmul(
                            po[:, 512:768],
                            aT[:, j, :],
                            wo[:, j, 512:768],
                            start=(j == 0),
                            stop=(j == ND - 1),
                        )
                    osb = p2s.tile([P, D], f32, tag="osb")
                    nc.vector.tensor_add(osb, po, ob)
                    nc.sync.dma_start(
                        out=out_d[ci * P : (ci + 1) * P, :], in_=osb
                    )

    nc.compile()
    return nc


def _host_prep(inputs):
    """Numpy-side constant preparation shared by all cores."""
    import ml_dtypes

    def f32(a):
        return np.asarray(a, dtype=np.float32)

    def wflat(W):  # [3, D, M] -> [D, 3*M] with feature index g*M + m
        return np.ascontiguousarray(
            np.transpose(f32(W), (1, 0, 2)).reshape(D, D)
        )

    def wchunks(Wf):  # [D, D] -> [P, ND, D] (contraction-chunk-major)
        return np.ascontiguousarray(Wf.reshape(ND, P, D).transpose(1, 0, 2))

    def brow(b):
        b2 = ((f32(b).reshape(D) + 0.5 * np.pi) / TWO_PI + BOFF).astype(np.float32)
        hi = b2.astype(ml_dtypes.bfloat16).astype(np.float32)
        lo = b2 - hi
        return np.ascontiguousarray(np.stack([hi, lo], axis=0))

    pos = np.arange(S, dtype=np.float32)
    dims = np.arange(0, HD, 2, dtype=np.float32)
    inv_freq = 1.0 / (np.float32(BASE) ** (dims / HD))
    ang = pos[:, None] * inv_freq[None, :]  # [S, 48]
    rc = (C * np.cos(ang)).astype(np.float32)
    rs = np.concatenate(
        [-C * np.sin(ang), C * np.sin(ang)], axis=1
    ).astype(np.float32)

    favor_W = f32(inputs["favor_W"])  # [96, 128]
    favor_b = f32(inputs["favor_b"]).reshape(R)
    b_hi = favor_b.astype(ml_dtypes.bfloat16).astype(np.float32)
    b_lo = favor_b - b_hi
    fwk = np.ascontiguousarray(
        np.concatenate([favor_W, b_hi[None, :], b_lo[None, :]], axis=0)
    )

    out_W = f32(inputs["out_W"])
    out_b = f32(inputs["out_b"]).reshape(D)

    common = dict(
        wq=wchunks(wflat(inputs["Wq"]) / TWO_PI),
        wk=wchunks(wflat(inputs["Wk"]) / TWO_PI),
        wv=wchunks(wflat(inputs["Wv"]) / TWO_PI),
        wo=wchunks(C * out_W),
        bq=brow(inputs["bq"]),
        bk=brow(inputs["bk"]),
        bv=brow(inputs["bv"]),
        ob=np.ascontiguousarray(np.broadcast_to(out_b, (P, D))),
        rc=rc,
        rs=rs,
        fw=favor_W,
        fwk=fwk,
        fb=np.ascontiguousarray(favor_b[:, None]),
    )
    return common


def kernel(**inputs) -> np.ndarray:
    from concourse import bass_utils

    if "nc" not in _cache:
        _cache["nc"] = _build(S, B)
    nc = _cache["nc"]

    common = _host_prep(inputs)
    x = np.asarray(inputs["x"], dtype=np.float32)
    in_maps = [
        dict(common, x=np.ascontiguousarray(x[i])) for i in range(B)
    ]
    res = bass_utils.run_bass_kernel_spmd(nc, in_maps, core_ids=list(range(B)))
    out = np.stack([res.results[i]["out"] for i in range(B)], axis=0)
    return out.astype(np.float32)


# revision 10
# speedup vs baseline: 1.2000x; 1.2000x over previous
"""Trainium2 Bass kernel: multi-head FAVOR+ attention with multi-scale
random-Fourier-feature inputs and rotary position embeddings.

Sharding: data-parallel over batch. Each of the 8 NeuronCores processes one
batch element end-to-end (every head's phi(q)/phi(k)/kv computation is
independent per batch element, so no collectives are needed); the host
scatters x over cores and stacks the per-core outputs.

Self-contained: hardcodes all shapes from the problem spec.
"""

import math

import numpy as np

B, S, D = 8, 4096, 768
H, HD = 8, 96
M = 256
R = 128
BASE = 12315
P = 128                  # tokens per chunk (= SBUF partitions)
NCH = S // P             # 32 chunks
ND = D // P              # 6 feature chunks of 128
GG = 4                   # chunks per ACT phase group (batches Sin/Exp tables)
TWO_PI = 2.0 * math.pi
C = math.sqrt(2.0 / M)   # multiscale feature scale
BOFF = 8.0               # positivity offset so round() reduction is exact
MAGIC = 8388608.0        # 2^23: (t + MAGIC) - MAGIC == round(t) for 0<t<2^22

_cache = {}


def _build(s_total, n_cores):
    """Build + compile the Bass module for one core processing [s_total, D]."""
    import concourse.bass as bass  # noqa: F401
    import concourse.tile as tile
    from concourse import bacc, mybir
    from concourse.masks import make_identity

    dt = mybir.dt
    AF = mybir.ActivationFunctionType
    ALU = mybir.AluOpType
    f32, bf16, fp16 = dt.float32, dt.bfloat16, dt.float16
    nch = s_total // P

    nc = bacc.Bacc(
        "TRN2", target_bir_lowering=False, debug=False, num_devices=n_cores
    )

    x_d = nc.dram_tensor("x", [s_total, D], f32, kind="ExternalInput").ap()
    wq_d = nc.dram_tensor("wq", [P, ND, D], f32, kind="ExternalInput").ap()
    wk_d = nc.dram_tensor("wk", [P, ND, D], f32, kind="ExternalInput").ap()
    wv_d = nc.dram_tensor("wv", [P, ND, D], f32, kind="ExternalInput").ap()
    wo_d = nc.dram_tensor("wo", [P, ND, D], f32, kind="ExternalInput").ap()
    bq_d = nc.dram_tensor("bq", [2, D], f32, kind="ExternalInput").ap()
    bk_d = nc.dram_tensor("bk", [2, D], f32, kind="ExternalInput").ap()
    bv_d = nc.dram_tensor("bv", [2, D], f32, kind="ExternalInput").ap()
    ob_d = nc.dram_tensor("ob", [P, D], f32, kind="ExternalInput").ap()
    rcf_d = nc.dram_tensor("rcf", [s_total, D], bf16, kind="ExternalInput").ap()
    rsf_d = nc.dram_tensor("rsf", [s_total, D], bf16, kind="ExternalInput").ap()
    fw_d = nc.dram_tensor("fw", [96, R], f32, kind="ExternalInput").ap()
    fwk_d = nc.dram_tensor("fwk", [98, R], f32, kind="ExternalInput").ap()
    fb_d = nc.dram_tensor("fb", [R, 1], f32, kind="ExternalInput").ap()
    out_d = nc.dram_tensor("out", [s_total, D], f32, kind="ExternalOutput").ap()

    with tile.TileContext(nc) as tc:
        with tc.tile_pool(name="const", bufs=1) as const:
            wq = const.tile([P, ND, D], bf16)
            nc.gpsimd.dma_start(out=wq, in_=wq_d)
            wk = const.tile([P, ND, D], bf16)
            nc.gpsimd.dma_start(out=wk, in_=wk_d)
            wv = const.tile([P, ND, D], bf16)
            nc.gpsimd.dma_start(out=wv, in_=wv_d)
            bq = const.tile([2, D], bf16)
            nc.gpsimd.dma_start(out=bq, in_=bq_d)
            bk = const.tile([2, D], bf16)
            nc.gpsimd.dma_start(out=bk, in_=bk_d)
            bv = const.tile([2, D], bf16)
            nc.gpsimd.dma_start(out=bv, in_=bv_d)
            ones1 = const.tile([2, P], bf16)
            nc.vector.memset(ones1, 1.0)
            ob = const.tile([P, D], f32)
            nc.sync.dma_start(out=ob, in_=ob_d)
            fw = const.tile([96, R], bf16)
            nc.gpsimd.dma_start(out=fw, in_=fw_d)
            fwk = const.tile([98, R], bf16)
            nc.gpsimd.dma_start(out=fwk, in_=fwk_d)
            fb = const.tile([R, 1], f32)
            nc.sync.dma_start(out=fb, in_=fb_d)
            idb = const.tile([P, P], bf16)
            make_identity(nc, idb)
            idf = const.tile([P, P], f32)
            make_identity(nc, idf)
            # resident q' features (feature-major) for pass 2
            qpT = const.tile([P, H, nch, P], bf16)
            kvaug = const.tile([P, H, 97], bf16)
            kvsb = const.tile([P, H * R], f32)
            # GG-way manually buffered transposed q/k tiles; kT rows 96:98
            # hold the ones that fold favor_b (hi/lo rows) into the matmul
            qTs, kTs = [], []
            for g in range(GG):
                qt = const.tile([P, H, P], bf16, tag=f"qT{g}")
                qTs.append(qt)
                kt = const.tile([P, H, P], bf16, tag=f"kT{g}")
                nc.vector.memset(kt[96:98, :, :], 1.0)
                kTs.append(kt)

            biases = {"q": bq, "k": bk, "v": bv}
            weights = {"q": wq, "k": wk, "v": wv}

            with (
                tc.tile_pool(name="w1", bufs=2) as w1,
                tc.tile_pool(name="wv1", bufs=GG + 1) as wv1,
                tc.tile_pool(name="wt1", bufs=3 * GG + 1) as wt1,
                tc.tile_pool(name="w2", bufs=3) as w2,
                tc.tile_pool(name="w3", bufs=2) as w3,
                tc.tile_pool(name="bigps", bufs=2, space="PSUM") as bigps,
                tc.tile_pool(name="tps", bufs=2, space="PSUM") as tps,
                tc.tile_pool(name="kvps", bufs=1, space="PSUM") as kvps,
            ):
                kv_ps = kvps.tile([P, H * R], f32)

                def front(ci):
                    """multiscale features + rotary + per-head transposes.
                    Emits Sin (no Exp) on the scalar engine."""
                    sl = slice(ci * P, (ci + 1) * P)
                    xb = w1.tile([P, D], bf16, tag="xb")
                    nc.gpsimd.dma_start(out=xb, in_=x_d[sl, :])
                    rcf = w1.tile([P, D], bf16, tag="rcf")
                    nc.sync.dma_start(out=rcf, in_=rcf_d[sl, :])
                    rsf = w1.tile([P, D], bf16, tag="rsf")
                    nc.sync.dma_start(out=rsf, in_=rsf_d[sl, :])

                    XT = w1.tile([P, ND, P], bf16, tag="XT")
                    for j in range(ND):
                        pt = tps.tile([P, P], bf16, tag="pt")
                        nc.tensor.transpose(pt, xb[:, j * P : (j + 1) * P], idb)
                        if j % 2 == 0:
                            nc.scalar.copy(XT[:, j, :], pt)
                        else:
                            nc.vector.tensor_copy(XT[:, j, :], pt)

                    feats = {}
                    for name in ("q", "k", "v"):
                        w_sb = weights[name]
                        pm = bigps.tile([P, H * R], f32, tag="big")
                        b_sb = biases[name]
                        nc.tensor.matmul(
                            pm[:, 0:512], ones1, b_sb[:, 0:512],
                            start=True, stop=False,
                        )
                        nc.tensor.matmul(
                            pm[:, 512:768], ones1, b_sb[:, 512:768],
                            start=True, stop=False,
                        )
                        for j in range(ND):
                            nc.tensor.matmul(
                                pm[:, 0:512],
                                XT[:, j, :],
                                w_sb[:, j, 0:512],
                                start=False,
                                stop=(j == ND - 1),
                            )
                            nc.tensor.matmul(
                                pm[:, 512:768],
                                XT[:, j, :],
                                w_sb[:, j, 512:768],
                                start=False,
                                stop=(j == ND - 1),
                            )
                        # r = t - round(t) in [-0.5, 0.5]; sin(2*pi*r) is the
                        # range-reduced multiscale cosine feature
                        t2 = w2.tile([P, D], fp16, tag="t2")
                        nc.vector.tensor_scalar(
                            out=t2, in0=pm[:, 0:D], scalar1=MAGIC, scalar2=MAGIC,
                            op0=ALU.add, op1=ALU.subtract,
                        )
                        t1 = wt1.tile([P, D], fp16, tag="t1")
                        nc.vector.tensor_sub(t1, pm[:, 0:D], t2)
                        if name == "v":
                            vaug = wv1.tile([P, H, 97], bf16, tag="vaug")
                            nc.scalar.activation(
                                vaug[:, :, 0:96],
                                t1.rearrange("p (h f) -> p h f", h=H),
                                AF.Sin,
                                bias=0.0,
                                scale=TWO_PI,
                            )
                            nc.vector.memset(vaug[:, :, 96:97], 1.0)
                            feats[name] = vaug
                        else:
                            ft = w1.tile([P, D], bf16, tag="f" + name)
                            nc.scalar.activation(
                                ft, t1, AF.Sin, bias=0.0, scale=TWO_PI
                            )
                            feats[name] = ft

                    # -0.5*||k_h||^2 (rotation-invariant: use pre-rotary feats)
                    k2 = w3.tile([P, D], bf16, tag="k2")
                    nc.vector.tensor_mul(k2, feats["k"], feats["k"])
                    nsq = wv1.tile([P, H], f32, tag="nsq")
                    nc.vector.reduce_sum(
                        nsq,
                        k2.rearrange("p (h f) -> p h f", h=H),
                        axis=mybir.AxisListType.X,
                    )
                    nc.vector.tensor_scalar_mul(nsq, nsq, -0.5 * C * C)

                    # rotary: ro = f*rcf + swap_t(f*rsf); the multipliers
                    # carry the sqrt(2/M) scale and the sign pattern
                    qt, kt = qTs[ci % GG], kTs[ci % GG]
                    for name, dst in (("q", qt), ("k", kt)):
                        f = feats[name]
                        ra = w3.tile([P, D], bf16, tag="ra")
                        nc.vector.tensor_mul(ra, f, rcf)
                        rb = w3.tile([P, D], bf16, tag="rb")
                        nc.vector.tensor_mul(rb, f, rsf)
                        ro = w1.tile([P, D], bf16, tag="ro" + name)
                        nc.vector.tensor_add(
                            ro.rearrange("p (h t f) -> p h t f", h=H, t=2),
                            ra.rearrange("p (h t f) -> p h t f", h=H, t=2),
                            rb.rearrange("p (h t f) -> p h t f", h=H, t=2)[
                                :, :, ::-1, :
                            ],
                        )
                        for h in range(H):
                            pt = tps.tile([P, P], bf16, tag="pt")
                            nc.tensor.transpose(
                                pt[0:96, :], ro[:, h * 96 : (h + 1) * 96], idb
                            )
                            if h % 2 == 0:
                                nc.scalar.copy(dst[0:96, h, :], pt[0:96, :])
                            else:
                                nc.vector.tensor_copy(
                                    dst[0:96, h, :], pt[0:96, :]
                                )
                    return feats["v"], nsq

                def back(ci, vaug, nsq):
                    """favor matmuls + Exp + kv accumulation."""
                    qt, kt = qTs[ci % GG], kTs[ci % GG]
                    # q' = exp(q.W + b); the exp(-0.5||q||^2) factor cancels
                    # in the final ratio (it scales numerator and z equally)
                    pfq = bigps.tile([P, H * R], f32, tag="big")
                    nc.tensor.matmul(
                        pfq[:, 0:512],
                        fw,
                        qt[0:96, 0:4, :].rearrange("p a b -> p (a b)"),
                        start=True,
                        stop=True,
                    )
                    nc.tensor.matmul(
                        pfq[:, 512:1024],
                        fw,
                        qt[0:96, 4:8, :].rearrange("p a b -> p (a b)"),
                        start=True,
                        stop=True,
                    )
                    nc.scalar.activation(
                        qpT[:, :, ci, :],
                        pfq.rearrange("p (h s) -> p h s", h=H),
                        AF.Exp,
                        bias=fb,
                        scale=1.0,
                    )

                    # k' = exp(k.W + b - 0.5||k||^2), token-major
                    pfk = bigps.tile([P, H * R], f32, tag="big")
                    for h in range(H):
                        nc.tensor.matmul(
                            pfk[:, h * R : (h + 1) * R],
                            kt[0:98, h, :],
                            fwk,
                            start=True,
                            stop=True,
                        )
                    pfk3 = pfk.rearrange("p (h r) -> p h r", h=H)
                    nc.vector.tensor_add(
                        pfk3, pfk3, nsq[:, :, None].broadcast_to([P, H, R])
                    )
                    kp = w1.tile([P, H, R], bf16, tag="kp")
                    nc.scalar.activation(kp, pfk3, AF.Exp)

                    # kv[r, hd] accumulation; v is augmented with a ones
                    # column so row 96 accumulates ksum[r]
                    for h in range(H):
                        nc.tensor.matmul(
                            kv_ps[0:97, h * R : (h + 1) * R],
                            vaug[:, h, :],
                            kp[:, h, :],
                            start=(ci == 0 and h % 4 == 0),
                            stop=(ci == nch - 1 and h % 4 == 3),
                        )

                for g0 in range(0, nch, GG):
                    saved = []
                    for ci in range(g0, min(g0 + GG, nch)):
                        saved.append((ci, *front(ci)))
                    for ci, vaug, nsq in saved:
                        back(ci, vaug, nsq)

                # [kv | ksum] -> [r, 97] per head, bf16
                nc.vector.tensor_copy(kvsb[0:97, :], kv_ps[0:97, :])
                for h in range(H):
                    pt = tps.tile([P, P], f32, tag="pt")
                    nc.tensor.transpose(
                        pt[:, 0:97],
                        kvsb[0:97, h * R : (h + 1) * R],
                        idf[0:97, 0:97],
                    )
                    if h % 2 == 0:
                        nc.scalar.copy(kvaug[:, h, :], pt[:, 0:97])
                    else:
                        nc.vector.tensor_copy(kvaug[:, h, :], pt[:, 0:97])

            # ---- pass 2: attention readout + output projection ----
            with (
                tc.tile_pool(name="p2c", bufs=1) as p2c,
                tc.tile_pool(name="p2s", bufs=2) as p2s,
                tc.tile_pool(name="atps", bufs=1, space="PSUM") as atps,
                tc.tile_pool(name="ops", bufs=2, space="PSUM") as ops,
                tc.tile_pool(name="tps2", bufs=2, space="PSUM") as tps2,
            ):
                wo = p2c.tile([P, ND, D], bf16)
                nc.gpsimd.dma_start(out=wo, in_=wo_d)
                for ci in range(nch):
                    pat = atps.tile([P, H * P], f32, tag="at")
                    for h in range(H):
                        nc.tensor.matmul(
                            pat[:, h * P : h * P + 97],
                            qpT[:, h, ci, :],
                            kvaug[:, h, :],
                            start=True,
                            stop=True,
                        )
                    pat3 = pat.rearrange("p (h r) -> p h r", h=H)
                    rz = p2s.tile([P, H], f32, tag="rz")
                    nc.vector.reciprocal(rz, pat3[:, :, 96])
                    at = p2s.tile([P, D], bf16, tag="at")
                    nc.vector.tensor_mul(
                        at.rearrange("p (h f) -> p h f", h=H),
                        pat3[:, :, 0:96],
                        rz[:, :, None].broadcast_to([P, H, 96]),
                    )
                    aT = p2s.tile([P, ND, P], bf16, tag="aT")
                    for j in range(ND):
                        pt = tps2.tile([P, P], bf16, tag="pt2")
                        nc.tensor.transpose(pt, at[:, j * P : (j + 1) * P], idb)
                        if j % 2 == 0:
                            nc.scalar.copy(aT[:, j, :], pt)
                        else:
                            nc.vector.tensor_copy(aT[:, j, :], pt)
                    po = ops.tile([P, D], f32, tag="po")
                    for j in range(ND):
                        nc.tensor.matmul(
                            po[:, 0:512],
                            aT[:, j, :],
                            wo[:, j, 0:512],
                            start=(j == 0),
                            stop=(j == ND - 1),
                        )
                        nc.tensor.matmul(
                            po[:, 512:768],
                            aT[:, j, :],
                            wo[:, j, 512:768],
                            start=(j == 0),
                            stop=(j == ND - 1),
                        )
                    osb = p2s.tile([P, D], f32, tag="osb")
                    nc.vector.tensor_add(osb, po, ob)
                    nc.sync.dma_start(
                        out=out_d[ci * P : (ci + 1) * P, :], in_=osb
                    )

    nc.compile()
    return nc


def _host_prep(inputs):
    """Numpy-side constant preparation shared by all cores."""
    import ml_dtypes

    def f32(a):
        return np.asarray(a, dtype=np.float32)

    def wflat(W):  # [3, D, M] -> [D, 3*M] with feature index g*M + m
        return np.ascontiguousarray(
            np.transpose(f32(W), (1, 0, 2)).reshape(D, D)
        )

    def wchunks(Wf):  # [D, D] -> [P, ND, D] (contraction-chunk-major)
        return np.ascontiguousarray(Wf.reshape(ND, P, D).transpose(1, 0, 2))

    def brow(b):
        b2 = ((f32(b).reshape(D) + 0.5 * np.pi) / TWO_PI + BOFF).astype(
            np.float32
        )
        hi = b2.astype(ml_dtypes.bfloat16).astype(np.float32)
        lo = b2 - hi
        return np.ascontiguousarray(np.stack([hi, lo], axis=0))

    pos = np.arange(S, dtype=np.float32)
    dims = np.arange(0, HD, 2, dtype=np.float32)
    inv_freq = 1.0 / (np.float32(BASE) ** (dims / HD))
    ang = pos[:, None] * inv_freq[None, :]          # [S, 48]
    csin, ccos = C * np.sin(ang), C * np.cos(ang)   # [S, 48]
    # rcf[s, (h,t,j)] = C*cos(ang[s,j]) for both halves t
    rcf = np.broadcast_to(
        ccos[:, None, None, :], (S, H, 2, 48)
    ).reshape(S, D)
    # rsf pre-swapped so that ro = f*rcf + swap_t(f*rsf) applies the
    # [x1*c - x2*s, x1*s + x2*c] rotation: rsf half t=0 holds +C*sin
    # (lands in t=1 after the swap), half t=1 holds -C*sin (lands in t=0)
    rsf = np.stack([csin, -csin], axis=1)           # [S, 2, 48]
    rsf = np.broadcast_to(rsf[:, None, :, :], (S, H, 2, 48)).reshape(S, D)

    favor_W = f32(inputs["favor_W"])                # [96, 128]
    favor_b = f32(inputs["favor_b"]).reshape(R)
    b_hi = favor_b.astype(ml_dtypes.bfloat16).astype(np.float32)
    b_lo = favor_b - b_hi
    fwk = np.ascontiguousarray(
        np.concatenate([favor_W, b_hi[None, :], b_lo[None, :]], axis=0)
    )

    out_W = f32(inputs["out_W"])
    out_b = f32(inputs["out_b"]).reshape(D)

    common = dict(
        wq=wchunks(wflat(inputs["Wq"]) / TWO_PI),
        wk=wchunks(wflat(inputs["Wk"]) / TWO_PI),
        wv=wchunks(wflat(inputs["Wv"]) / TWO_PI),
        wo=wchunks(C * out_W),
        bq=brow(inputs["bq"]),
        bk=brow(inputs["bk"]),
        bv=brow(inputs["bv"]),
        ob=np.ascontiguousarray(np.broadcast_to(out_b, (P, D))),
        rcf=np.ascontiguousarray(rcf.astype(ml_dtypes.bfloat16)),
        rsf=np.ascontiguousarray(rsf.astype(ml_dtypes.bfloat16)),
        fw=favor_W,
        fwk=fwk,
        fb=np.ascontiguousarray(favor_b[:, None]),
    )
    return common


def kernel(**inputs) -> np.ndarray:
    from concourse import bass_utils

    if "nc" not in _cache:
        _cache["nc"] = _build(S, B)
    nc = _cache["nc"]

    common = _host_prep(inputs)
    x = np.asarray(inputs["x"], dtype=np.float32)
    in_maps = [
        dict(common, x=np.ascontiguousarray(x[i])) for i in range(B)
    ]
    res = bass_utils.run_bass_kernel_spmd(nc, in_maps, core_ids=list(range(B)))
    out = np.stack([res.results[i]["out"] for i in range(B)], axis=0)
    return out.astype(np.float32)
